# revision 10
# baseline (speedup 1.0000x reference)
"""Multi-head self-attention (B=4, T=2048, C=1024, 16 heads x hd=64) on 8
Trainium2 NeuronCores.

Sharding: tensor-parallel over heads - each core owns 2 heads (128 of the
1024 channels). Every core reads the full x (transposed + bf16-cast on host),
computes Q^T/K^T (channel-major) and V (token-major, fp8 + fp8 residual),
runs attention from SBUF, and produces a rank-128 partial of the output
projection. The 8 fp16 partials are summed on host together with the
constant bias vector bv @ Wo^T + bo (softmax rows sum to 1, so bv's
contribution to the output is constant and never touches the device).

Per-core engine split (the kernel is activation-throughput limited):
  PE:    projections (bf16), S^T as row-tiled 64x128 matmul pairs (the two
         heads run CONCURRENTLY in the top/bottom halves of the PE array),
         PV as fp8 DoubleRow matmuls (2 k-tiles of 128 keys per matmul),
         out-projection (bf16).
  ACT:   ~10/16 of the exponentials (true Exp, fp32 PSUM -> fp8e4 SBUF,
         scale=1/8 folded), Q/K projection evictions, half the p3 evictions.
  DVE:   ~6/16 of the exponentials via the Schraudolph bit trick
         (u8 = s/ln2 + 55.5, bit pattern IS e4m3 exp(s/8)), V quantize +
         fp8 residual, PV eviction, reciprocal, half of p3 evictions.
  Pool:  epilogue combines (V + residual), softmax normalize, small DMAs.

The V error is nearly eliminated by storing res = V - fp8(V) in the spare
columns of the PV stationary (cols 65..127, 63 of 64 channels) and adding
the resulting correction rows into O during the epilogue (shift-by-64
realignment via a tiny SBUF->SBUF DMA).
"""
import json

import numpy as np
import ml_dtypes

import concourse.bass as bass
import concourse.mybir as mybir
import concourse.tile as tile
from concourse.bass_utils import run_bass_kernel_spmd

bf16 = ml_dtypes.bfloat16
dt = mybir.dt

EMB = 1024
HEADS = 16
HD = 64
B = 4
T = 2048
R = B * T            # 8192 rows
NCORES = 8
F = EMB // NCORES    # 128 channels (2 heads) per core
NH = F // HD         # 2 heads per core
NKC = EMB // 128     # 8 contraction chunks for projections
NKT = T // 128       # 16 key tiles per batch
NQB = T // 512       # 4 query blocks per batch
G = R // 128         # 64 global row/key tiles
VW = 128             # va slot: 64 V | 1 ones | 63 residual
NRING = 8            # pt ring slots (1024 fp8 each)

# kt%16 values whose exp runs on DVE (Schraudolph); rest on ACT (true exp)
DVE_KT = (1, 4, 6, 9, 12, 14)
SCHR_A = 1.4426950408889634     # 1/ln2: folds the 1/8 score scale into e4m3 bits
SCHR_B = 55.54                  # 56 (e4m3 bias*8) - 0.46 rounding/Schraudolph tune


# ---------------------------------------------------------------------------
# walrus in this container accepts only ONE sync-wait per instruction; split
# extra waits onto same-engine NoOps at BIR-serialization time.
_orig_to_json_bytes = bass.Bass.to_json_bytes


def _split_waits(data: bytes) -> bytes:
    d = json.loads(data)
    changed = False
    for f in d.get("functions", []):
        for blk in f.get("blocks", []):
            out = []
            for inst in blk.get("instructions", []):
                si = inst.get("sync_info")
                waits = (si or {}).get("on_wait") or []
                if len(waits) > 1:
                    changed = True
                    for i, w in enumerate(waits[:-1]):
                        out.append({
                            "debug": inst.get("debug", 0),
                            "engine": inst["engine"],
                            "ins": [], "outs": [],
                            "name": f"{inst['name']}_w{i}",
                            "opcode": "NoOp",
                            "sync_info": {"on_update": [], "on_wait": [w]},
                            "text_hint": "wait_split",
                        })
                    si["on_wait"] = waits[-1:]
                out.append(inst)
            blk["instructions"] = out
    return json.dumps(d).encode() if changed else data


def _to_json_bytes(self, *a, **k):
    return _split_waits(_orig_to_json_bytes(self, *a, **k))


bass.Bass.to_json_bytes = _to_json_bytes
# ---------------------------------------------------------------------------


def build_bass() -> bass.Bass:
    nc = bass.Bass()
    xt_ext = nc.declare_dram_parameter("xt", [16 * 128, NKC * 512], dt.bfloat16,
                                   isOutput=False)
    wq_ext = nc.declare_dram_parameter("wq", [EMB, F], dt.bfloat16, isOutput=False)
    wk_ext = nc.declare_dram_parameter("wk", [EMB, F], dt.bfloat16, isOutput=False)
    wv_ext = nc.declare_dram_parameter("wv", [EMB, F], dt.bfloat16, isOutput=False)
    wo_ext = nc.declare_dram_parameter("wo", [F, EMB], dt.bfloat16, isOutput=False)
    bq_ext = nc.declare_dram_parameter("bq", [F, 1], dt.float32, isOutput=False)
    out_ext = nc.declare_dram_parameter("out", [R, EMB], dt.float16, isOutput=True)

    Exp = mybir.ActivationFunctionType.Exp
    Idn = mybir.ActivationFunctionType.Identity
    Cpy = mybir.ActivationFunctionType.Copy
    DR = mybir.MatmulPerfMode.DoubleRow
    Alu = mybir.AluOpType

    with tile.TileContext(nc) as tc:
        with (
            tc.tile_pool(name="const", bufs=1) as cp,
            tc.tile_pool(name="res", bufs=1) as res,
            tc.tile_pool(name="xt", bufs=4) as xp,
            tc.tile_pool(name="norm", bufs=2) as npl,
            tc.tile_pool(name="osb", bufs=4) as op,
            tc.tile_pool(name="ps", bufs=1, space="PSUM") as ps,
        ):
            # --- constants ---
            wq_sb = cp.tile([128, EMB], dt.bfloat16, tag="wq")
            wk_sb = cp.tile([128, EMB], dt.bfloat16, tag="wk")
            wv_sb = cp.tile([128, EMB], dt.bfloat16, tag="wv")
            wo_sb = cp.tile([128, EMB], dt.bfloat16, tag="wo")
            bq_sb = cp.tile([F, 1], dt.float32, tag="bq")

            def _wload(ext, tile_sb):
                nc.sync.dma_start(
                    tile_sb[:].rearrange("p (kc f) -> p kc f", f=F),
                    ext[:].rearrange("(kc p) f -> p kc f", p=128),
                )
            _wload(wq_ext, wq_sb)
            nc.sync.dma_start(bq_sb[:], bq_ext[:])
            _wload(wk_ext, wk_sb)
            _wload(wv_ext, wv_sb)
            nc.sync.dma_start(wo_sb[:], wo_ext[:])

            # --- residents ---
            qt_sb = res.tile([F, R], dt.bfloat16, tag="qt")
            kt_sb = res.tile([F, R], dt.bfloat16, tag="kt")
            ot_sb = res.tile([F, R], dt.bfloat16, tag="ot")
            va_sb = res.tile([128, G * NH * VW], dt.float8e4, tag="va")
            pt_sb = res.tile([128, NRING * 1024], dt.float8e4, tag="pt")
            nc.vector.memset(
                va_sb[:].rearrange("p (g w) -> p g w", w=VW)[:, :, HD:HD + 1], 1.0
            )

            def va_view():
                return va_sb[:].rearrange("p (g h w) -> p g h w", h=NH, w=VW)

            def pt_view():
                return pt_sb[:].rearrange("p (s h q) -> p s h q", h=NH, q=512)

            # ---- phase-1 emitters (one 512-row block = 7 filler units) ----
            def p1_load(rb):
                xt = xp.tile([128, NKC * 512], dt.bfloat16, tag="xt",
                             name=f"xt_{rb}")
                nc.sync.dma_start(xt[:], xt_ext[rb * 128:(rb + 1) * 128, :])
                return xt

            def p1_qk(rb, xt, w_sb, dst_sb, bias):
                """Returns the deferred eviction closure."""
                r0 = rb * 512
                xtv = xt[:].rearrange("p (kc r) -> p kc r", kc=NKC)
                acc = ps.tile([128, 512], dt.float32, tag="pp", bufs=2,
                              name=f"prj_{rb}_{id(w_sb)}")[:]
                for kc in range(NKC):
                    nc.tensor.matmul(
                        acc, w_sb[:, kc * F:(kc + 1) * F], xtv[:, kc, :],
                        start=(kc == 0), stop=(kc == NKC - 1),
                    )

                def evict():
                    if bias is not None:
                        nc.scalar.activation(dst_sb[:, r0:r0 + 512], acc, Idn,
                                             bias=bias[:])
                    else:
                        nc.scalar.activation(dst_sb[:, r0:r0 + 512], acc, Cpy)
                return evict

            def p1_v(rb, xt, sub):
                """Returns the deferred eviction closure."""
                g = rb * 4 + sub
                xtv = xt[:].rearrange("p (kc r) -> p kc r", kc=NKC)
                acc = ps.tile([128, F], dt.float32, tag="pp", bufs=2,
                              name=f"vprj_{g}")[:]
                for kc in range(NKC):
                    nc.tensor.matmul(
                        acc,
                        xtv[:, kc, sub * 128:(sub + 1) * 128],
                        wv_sb[:, kc * F:(kc + 1) * F],
                        start=(kc == 0), stop=(kc == NKC - 1),
                    )

                def evict():
                    vav = va_view()
                    # fp8 main V
                    nc.vector.tensor_copy(
                        vav[:, g, :, 0:HD],
                        acc.rearrange("p (h d) -> p h d", d=HD),
                    )
                    # fp8 residual, channels 0..62 per head: res = acc - fp8(V)
                    nc.vector.scalar_tensor_tensor(
                        vav[:, g, :, HD + 1:VW],
                        acc.rearrange("p (h d) -> p h d", d=HD)[:, :, 0:HD - 1],
                        1.0,
                        vav[:, g, :, 0:HD - 1],
                        op0=Alu.mult, op1=Alu.subtract,
                    )
                return evict

            def p1_block_fillers(rb):
                state = {}

                def load():
                    state["xt"] = p1_load(rb)

                return [load,
                        lambda: p1_qk(rb, state["xt"], wq_sb, qt_sb, bq_sb),
                        lambda: p1_qk(rb, state["xt"], wk_sb, kt_sb, None),
                        lambda: p1_v(rb, state["xt"], 0),
                        lambda: p1_v(rb, state["xt"], 1),
                        lambda: p1_v(rb, state["xt"], 2),
                        lambda: p1_v(rb, state["xt"], 3)]

            # Deferred PSUM evictions: each filler's matmuls are emitted at
            # one fill point but its ACT/DVE eviction is emitted at the NEXT
            # point, so evictions never sit ahead of ready exps in the
            # in-order engine queues with unresolved PE dependencies.
            deferred = []

            def emit_unit(f):
                while len(deferred) >= 2:   # pp pool has 2 slots
                    deferred.pop(0)()
                ev = f()
                if ev is not None:
                    deferred.append(ev)

            def flush_deferred():
                while deferred:
                    deferred.pop(0)()

            # ---- phase-3 emitter (one 128-row tile) ----
            def p3_tile(g):
                """Returns the deferred eviction closure."""
                o_sb = op.tile([128, EMB], dt.float16, tag="osb", name=f"o_{g}")
                accs = []
                for ch in range(2):
                    o_ps = ps.tile([128, 512], dt.float32, tag="pp", bufs=2,
                                   name=f"ops_{g}_{ch}")[:]
                    nc.tensor.matmul(
                        o_ps,
                        ot_sb[:, g * 128:(g + 1) * 128],
                        wo_sb[:, ch * 512:(ch + 1) * 512],
                        start=True, stop=True,
                    )
                    accs.append(o_ps)

                def evict():
                    nc.scalar.activation(o_sb[:, 0:512], accs[0], Cpy)
                    nc.vector.tensor_copy(o_sb[:, 512:1024], accs[1])
                    eng = nc.sync if g % 2 == 0 else nc.gpsimd
                    eng.dma_start(out_ext[g * 128:(g + 1) * 128, :], o_sb[:])
                return evict

            # ---- phase-2 q-block ----
            def p2_qblock(b, qb, fillers):
                q0 = b * T + qb * 512
                fi = iter(fillers)

                def fill(n=1):
                    flush_deferred()
                    for _ in range(n):
                        f = next(fi, None)
                        if f is not None:
                            emit_unit(f)

                pvs = {h: ps.tile([128, 512], dt.float32, tag="pv", bufs=2,
                                  name=f"pv_{b}_{qb}_{h}")
                       for h in range(NH)}
                ptv = pt_view()

                def emit_pv(pr):
                    # PV (DoubleRow fp8, 2 k-tiles per matmul) for pair pr
                    g0 = b * NKT + 2 * pr
                    s0 = (qb * NKT + 2 * pr) % NRING
                    for h in range(NH):
                        nc.tensor.matmul(
                            pvs[h][:],
                            va_view()[:, g0:g0 + 2, h, :],
                            ptv[:, s0:s0 + 2, h, :],
                            start=(pr == 0), stop=(pr == 7),
                            perf_mode=DR,
                        )

                # Software pipeline in groups of 2 k-tiles: the PE stream per
                # group is [S-pair, S-pair, PV(prev pair), filler] so the
                # S^T matmuls (which everything downstream waits on) always
                # lead, and PV lags one pair behind the exps it consumes.
                for grp in range(8):
                    for k2 in range(2):
                        kt = 2 * grp + k2
                        k0 = b * T + kt * 128
                        st = ps.tile([128, 1024], dt.float32, tag="st", bufs=2,
                                     name=f"st_{b}_{qb}_{kt}")
                        for h in range(NH):
                            nc.tensor.matmul(
                                st[:, h * 512:(h + 1) * 512],
                                kt_sb[h * HD:(h + 1) * HD, k0:k0 + 128],
                                qt_sb[h * HD:(h + 1) * HD, q0:q0 + 512],
                                start=True, stop=True,
                            )
                        slot = (qb * NKT + kt) % NRING
                        pts = pt_sb[:, slot * 1024:(slot + 1) * 1024]
                        if (kt % 16) in DVE_KT:
                            nc.vector.tensor_scalar(
                                pts.bitcast(dt.uint8), st[:],
                                SCHR_A, SCHR_B, op0=Alu.mult, op1=Alu.add,
                            )
                        else:
                            nc.scalar.activation(pts, st[:], Exp, scale=0.125)
                    if grp > 0:
                        emit_pv(grp - 1)
                    fill(2 if grp % 2 == 0 else 1)
                emit_pv(7)
                fill(100)   # drain leftover filler units

                # epilogue stage A (immediate): evacuate pv PSUM, start the
                # denominator-reshape and residual-shift DMAs
                stA = []
                for h in range(NH):
                    pv = pvs[h]
                    m = npl.tile([128, 512], dt.float32, tag="m",
                                 name=f"m_{b}_{qb}_{h}")
                    nc.vector.tensor_copy(m[:], pv[:])
                    ms = npl.tile([HD, 512], dt.float32, tag="ms",
                                  name=f"ms_{b}_{qb}_{h}")
                    nc.gpsimd.memset(ms[:], 0.0)
                    nc.gpsimd.dma_start(ms[0:HD - 1, :], m[HD + 1:128, :])
                    stA.append((h, m, ms))

                # stage B (deferred into the next q-block so the d4->recip->
                # broadcast DMA latency never blocks the next exp stream)
                def stage_b(items=stA, b=b, qb=qb, q0=q0):
                    for h, m, ms in items:
                        rc = npl.tile([1, 512], dt.float32, tag="rc",
                                      name=f"rc_{b}_{qb}_{h}")
                        nc.vector.reciprocal(rc[:], m[HD:HD + 1, :])
                        rbt = npl.tile([HD, 512], dt.float32, tag="rb",
                                       name=f"rb_{b}_{qb}_{h}")
                        nc.gpsimd.dma_start(
                            rbt[:],
                            rc[0:1, :].rearrange("p (o q) -> p o q", o=1)
                            .broadcast_to((1, HD, 512)),
                        )
                        otm = npl.tile([HD, 512], dt.float32, tag="otm",
                                       name=f"otm_{b}_{qb}_{h}")
                        nc.gpsimd.tensor_add(otm[:], m[0:HD, :], ms[:])
                        osl = ot_sb[h * HD:(h + 1) * HD, q0:q0 + 512]
                        nc.gpsimd.tensor_mul(osl, otm[:], rbt[:])

                if b == B - 1 and qb == NQB - 1:
                    stage_b()
                else:
                    deferred.append(stage_b)

            # ---------------- emission schedule ----------------
            for rb in range(4):
                for f in p1_block_fillers(rb):
                    emit_unit(f)
            flush_deferred()
            for b in range(B):
                for qb in range(NQB):
                    fillers = []
                    if b < B - 1:
                        fillers.extend(p1_block_fillers((b + 1) * 4 + qb))
                    if b > 0:
                        for g in range((b - 1) * 16 + qb * 4,
                                       (b - 1) * 16 + qb * 4 + 4):
                            fillers.append(lambda g=g: p3_tile(g))
                    if b == B - 1 and qb > 0:
                        for g in range(48 + (qb - 1) * 4, 48 + qb * 4):
                            fillers.append(lambda g=g: p3_tile(g))
                    p2_qblock(b, qb, fillers)
            for g in range(60, 64):
                emit_unit(lambda g=g: p3_tile(g))
            flush_deferred()
    return nc


_NC_CACHE = None


def _get_nc():
    global _NC_CACHE
    if _NC_CACHE is None:
        _NC_CACHE = build_bass()
    return _NC_CACHE


def make_in_maps(x, Wq, bq, Wk, bk, Wv, bv, Wo, bo):
    xt = np.asarray(x, dtype=np.float32).reshape(R, EMB).astype(bf16).T
    # partition-major tiles: [rb, p, kc, r] so each SBUF load is 128
    # contiguous 8KB descriptors instead of 1024 x 1KB fragments
    xt = np.ascontiguousarray(
        xt.reshape(NKC, 128, 16, 512).transpose(2, 1, 0, 3)
    ).reshape(16 * 128, NKC * 512)
    in_maps = []
    for c in range(NCORES):
        rows = slice(F * c, F * (c + 1))
        in_maps.append({
            "xt": xt,
            "wq": np.ascontiguousarray(np.asarray(Wq)[rows, :].T.astype(bf16)),
            "wk": np.ascontiguousarray(np.asarray(Wk)[rows, :].T.astype(bf16)),
            "wv": np.ascontiguousarray(np.asarray(Wv)[rows, :].T.astype(bf16)),
            "wo": np.ascontiguousarray(np.asarray(Wo)[:, rows].T.astype(bf16)),
            "bq": np.asarray(bq)[rows].reshape(F, 1).astype(np.float32),
        })
    return in_maps


def gather(results, Wv, bv, Wo, bo):
    acc = np.zeros((R, EMB), np.float32)
    for r in results:
        acc += r["out"].astype(np.float32)
    # softmax rows sum to 1, so the V-bias contributes the constant bv @ Wo^T
    acc += (np.asarray(bv, np.float64) @ np.asarray(Wo, np.float64).T
            + np.asarray(bo, np.float64)).astype(np.float32)
    return acc.reshape(B, T, EMB)


def kernel(x, Wq, bq, Wk, bk, Wv, bv, Wo, bo, _trace=False):
    nc = _get_nc()
    in_maps = make_in_maps(x, Wq, bq, Wk, bk, Wv, bv, Wo, bo)
    res = run_bass_kernel_spmd(nc, in_maps, list(range(NCORES)), trace=_trace)
    out = gather(res.results, Wv, bv, Wo, bo)
    if _trace:
        kernel.last_result = res
    return out


# revision 11
# speedup vs baseline: 1.0816x; 1.0816x over previous
"""Multi-head self-attention (B=4, T=2048, C=1024, 16 heads x hd=64) on 8
Trainium2 NeuronCores.

Sharding: tensor-parallel over heads - each core owns 2 heads (128 of the
1024 channels). Every core reads the full x (transposed + bf16-cast on host),
computes Q^T/K^T (channel-major) and V (token-major, fp8 + fp8 residual),
runs attention from SBUF, and produces a rank-128 partial of the output
projection. The 8 fp16 partials are summed on host together with the
constant bias vector bv @ Wo^T + bo (softmax rows sum to 1, so bv's
contribution to the output is constant and never touches the device).

Per-core engine split (the kernel is activation-throughput limited):
  PE:    projections (bf16), S^T as row-tiled 64x128 matmul pairs (the two
         heads run CONCURRENTLY in the top/bottom halves of the PE array),
         PV as fp8 DoubleRow matmuls (2 k-tiles of 128 keys per matmul),
         out-projection (bf16).
  ACT:   ~10/16 of the exponentials (true Exp, fp32 PSUM -> fp8e4 SBUF,
         scale=1/8 folded), Q/K projection evictions, half the p3 evictions.
  DVE:   ~6/16 of the exponentials via the Schraudolph bit trick
         (u8 = s/ln2 + 55.5, bit pattern IS e4m3 exp(s/8)), V quantize +
         fp8 residual, PV eviction, reciprocal, half of p3 evictions.
  Pool:  epilogue combines (V + residual), softmax normalize, small DMAs.

The V error is nearly eliminated by storing res = V - fp8(V) in the spare
columns of the PV stationary (cols 65..127, 63 of 64 channels) and adding
the resulting correction rows into O during the epilogue (shift-by-64
realignment via a tiny SBUF->SBUF DMA).
"""
import json

import numpy as np
import ml_dtypes

import concourse.bass as bass
import concourse.mybir as mybir
import concourse.tile as tile
from concourse.bass_utils import run_bass_kernel_spmd

bf16 = ml_dtypes.bfloat16
dt = mybir.dt

EMB = 1024
HEADS = 16
HD = 64
B = 4
T = 2048
R = B * T            # 8192 rows
NCORES = 8
F = EMB // NCORES    # 128 channels (2 heads) per core
NH = F // HD         # 2 heads per core
NKC = EMB // 128     # 8 contraction chunks for projections
NKT = T // 128       # 16 key tiles per batch
NQB = T // 512       # 4 query blocks per batch
G = R // 128         # 64 global row/key tiles
VW = 128             # va slot: 64 V | 1 ones | 63 residual
NRING = 8            # pt ring slots (1024 fp8 each)

# kt%16 values whose exp runs on DVE (Schraudolph); rest on ACT (true exp)
DVE_KT = (1, 4, 6, 9, 12, 14)
SCHR_A = 1.4426950408889634     # 1/ln2: folds the 1/8 score scale into e4m3 bits
SCHR_B = 55.54                  # 56 (e4m3 bias*8) - 0.46 rounding/Schraudolph tune


# ---------------------------------------------------------------------------
# walrus in this container accepts only ONE sync-wait per instruction; split
# extra waits onto same-engine NoOps at BIR-serialization time.
_orig_to_json_bytes = bass.Bass.to_json_bytes


def _split_waits(data: bytes) -> bytes:
    d = json.loads(data)
    changed = False
    for f in d.get("functions", []):
        for blk in f.get("blocks", []):
            out = []
            for inst in blk.get("instructions", []):
                si = inst.get("sync_info")
                waits = (si or {}).get("on_wait") or []
                if len(waits) > 1:
                    changed = True
                    for i, w in enumerate(waits[:-1]):
                        out.append({
                            "debug": inst.get("debug", 0),
                            "engine": inst["engine"],
                            "ins": [], "outs": [],
                            "name": f"{inst['name']}_w{i}",
                            "opcode": "NoOp",
                            "sync_info": {"on_update": [], "on_wait": [w]},
                            "text_hint": "wait_split",
                        })
                    si["on_wait"] = waits[-1:]
                out.append(inst)
            blk["instructions"] = out
    return json.dumps(d).encode() if changed else data


def _to_json_bytes(self, *a, **k):
    return _split_waits(_orig_to_json_bytes(self, *a, **k))


bass.Bass.to_json_bytes = _to_json_bytes
# ---------------------------------------------------------------------------


def build_bass() -> bass.Bass:
    nc = bass.Bass()
    xt_ext = nc.declare_dram_parameter("xt", [16 * 128, NKC * 512], dt.bfloat16,
                                   isOutput=False)
    wq_ext = nc.declare_dram_parameter("wq", [EMB, F], dt.bfloat16, isOutput=False)
    wk_ext = nc.declare_dram_parameter("wk", [EMB, F], dt.bfloat16, isOutput=False)
    wv_ext = nc.declare_dram_parameter("wv", [EMB, F], dt.bfloat16, isOutput=False)
    wo_ext = nc.declare_dram_parameter("wo", [F, EMB], dt.bfloat16, isOutput=False)
    bq_ext = nc.declare_dram_parameter("bq", [F, 1], dt.float32, isOutput=False)
    out_ext = nc.declare_dram_parameter("out", [R, EMB], dt.float16, isOutput=True)

    Exp = mybir.ActivationFunctionType.Exp
    Idn = mybir.ActivationFunctionType.Identity
    Cpy = mybir.ActivationFunctionType.Copy
    DR = mybir.MatmulPerfMode.DoubleRow
    Alu = mybir.AluOpType

    with tile.TileContext(nc) as tc:
        with (
            tc.tile_pool(name="const", bufs=1) as cp,
            tc.tile_pool(name="res", bufs=1) as res,
            tc.tile_pool(name="xt", bufs=4) as xp,
            tc.tile_pool(name="norm", bufs=2) as npl,
            tc.tile_pool(name="osb", bufs=4) as op,
            tc.tile_pool(name="ps", bufs=1, space="PSUM") as ps,
        ):
            # --- constants ---
            wq_sb = cp.tile([128, EMB], dt.bfloat16, tag="wq")
            wk_sb = cp.tile([128, EMB], dt.bfloat16, tag="wk")
            wv_sb = cp.tile([128, EMB], dt.bfloat16, tag="wv")
            wo_sb = cp.tile([128, EMB], dt.bfloat16, tag="wo")
            bq_sb = cp.tile([F, 1], dt.float32, tag="bq")

            def _wload(ext, tile_sb):
                nc.sync.dma_start(
                    tile_sb[:].rearrange("p (kc f) -> p kc f", f=F),
                    ext[:].rearrange("(kc p) f -> p kc f", p=128),
                )
            _wload(wq_ext, wq_sb)
            nc.sync.dma_start(bq_sb[:], bq_ext[:])
            _wload(wk_ext, wk_sb)
            _wload(wv_ext, wv_sb)
            nc.sync.dma_start(wo_sb[:], wo_ext[:])

            # --- residents ---
            qt_sb = res.tile([F, R], dt.bfloat16, tag="qt")
            kt_sb = res.tile([F, R], dt.bfloat16, tag="kt")
            ot_sb = res.tile([F, R], dt.bfloat16, tag="ot")
            va_sb = res.tile([128, G * NH * VW], dt.float8e4, tag="va")
            pt_sb = res.tile([128, NRING * 1024], dt.float8e4, tag="pt")
            nc.vector.memset(
                va_sb[:].rearrange("p (g w) -> p g w", w=VW)[:, :, HD:HD + 1], 1.0
            )

            def va_view():
                return va_sb[:].rearrange("p (g h w) -> p g h w", h=NH, w=VW)

            def pt_view():
                return pt_sb[:].rearrange("p (s h q) -> p s h q", h=NH, q=512)

            # ---- phase-1 emitters (one 512-row block = 7 filler units) ----
            def p1_load(rb):
                xt = xp.tile([128, NKC * 512], dt.bfloat16, tag="xt",
                             name=f"xt_{rb}")
                nc.sync.dma_start(xt[:], xt_ext[rb * 128:(rb + 1) * 128, :])
                return xt

            def p1_qk(rb, xt, w_sb, dst_sb, bias):
                """Returns the deferred eviction closure."""
                r0 = rb * 512
                xtv = xt[:].rearrange("p (kc r) -> p kc r", kc=NKC)
                acc = ps.tile([128, 512], dt.float32, tag="pp", bufs=2,
                              name=f"prj_{rb}_{id(w_sb)}")[:]
                for kc in range(NKC):
                    nc.tensor.matmul(
                        acc, w_sb[:, kc * F:(kc + 1) * F], xtv[:, kc, :],
                        start=(kc == 0), stop=(kc == NKC - 1),
                    )

                def evict():
                    if bias is not None:
                        nc.scalar.activation(dst_sb[:, r0:r0 + 512], acc, Idn,
                                             bias=bias[:])
                    else:
                        nc.scalar.activation(dst_sb[:, r0:r0 + 512], acc, Cpy)
                return evict

            def p1_v(rb, xt, sub):
                """Returns the deferred eviction closure."""
                g = rb * 4 + sub
                xtv = xt[:].rearrange("p (kc r) -> p kc r", kc=NKC)
                acc = ps.tile([128, F], dt.float32, tag="pp", bufs=2,
                              name=f"vprj_{g}")[:]
                for kc in range(NKC):
                    nc.tensor.matmul(
                        acc,
                        xtv[:, kc, sub * 128:(sub + 1) * 128],
                        wv_sb[:, kc * F:(kc + 1) * F],
                        start=(kc == 0), stop=(kc == NKC - 1),
                    )

                def evict():
                    vav = va_view()
                    # fp8 main V
                    nc.vector.tensor_copy(
                        vav[:, g, :, 0:HD],
                        acc.rearrange("p (h d) -> p h d", d=HD),
                    )
                    # fp8 residual, channels 0..62 per head: res = acc - fp8(V)
                    nc.vector.scalar_tensor_tensor(
                        vav[:, g, :, HD + 1:VW],
                        acc.rearrange("p (h d) -> p h d", d=HD)[:, :, 0:HD - 1],
                        1.0,
                        vav[:, g, :, 0:HD - 1],
                        op0=Alu.mult, op1=Alu.subtract,
                    )
                return evict

            def p1_block_fillers(rb):
                state = {}

                def load():
                    state["xt"] = p1_load(rb)

                return [load,
                        lambda: p1_qk(rb, state["xt"], wq_sb, qt_sb, bq_sb),
                        lambda: p1_qk(rb, state["xt"], wk_sb, kt_sb, None),
                        lambda: p1_v(rb, state["xt"], 0),
                        lambda: p1_v(rb, state["xt"], 1),
                        lambda: p1_v(rb, state["xt"], 2),
                        lambda: p1_v(rb, state["xt"], 3)]

            # Deferred PSUM evictions: each filler's matmuls are emitted at
            # one fill point but its ACT/DVE eviction is emitted at the NEXT
            # point, so evictions never sit ahead of ready exps in the
            # in-order engine queues with unresolved PE dependencies.
            deferred = []

            def emit_unit(f):
                while len(deferred) >= 2:   # pp pool has 2 slots
                    deferred.pop(0)()
                ev = f()
                if ev is not None:
                    deferred.append(ev)

            def flush_deferred():
                while deferred:
                    deferred.pop(0)()

            # ---- phase-3 emitter (one 128-row tile) ----
            def p3_tile(g):
                """Returns the deferred eviction closure."""
                o_sb = op.tile([128, EMB], dt.float16, tag="osb", name=f"o_{g}")
                accs = []
                for ch in range(2):
                    o_ps = ps.tile([128, 512], dt.float32, tag="pp", bufs=2,
                                   name=f"ops_{g}_{ch}")[:]
                    nc.tensor.matmul(
                        o_ps,
                        ot_sb[:, g * 128:(g + 1) * 128],
                        wo_sb[:, ch * 512:(ch + 1) * 512],
                        start=True, stop=True,
                    )
                    accs.append(o_ps)

                def evict():
                    nc.scalar.activation(o_sb[:, 0:512], accs[0], Cpy)
                    nc.vector.tensor_copy(o_sb[:, 512:1024], accs[1])
                    eng = nc.sync if g % 2 == 0 else nc.gpsimd
                    eng.dma_start(out_ext[g * 128:(g + 1) * 128, :], o_sb[:])
                return evict

            # ---- phase-2 q-block ----
            def p2_qblock(b, qb, fillers):
                q0 = b * T + qb * 512
                fi = iter(fillers)

                def fill(n=1):
                    flush_deferred()
                    for _ in range(n):
                        f = next(fi, None)
                        if f is not None:
                            emit_unit(f)

                pvs = {h: ps.tile([128, 512], dt.float32, tag="pv", bufs=2,
                                  name=f"pv_{b}_{qb}_{h}")
                       for h in range(NH)}
                ptv = pt_view()

                def emit_pv(pr):
                    # PV (DoubleRow fp8, 2 k-tiles per matmul) for pair pr
                    g0 = b * NKT + 2 * pr
                    s0 = (qb * NKT + 2 * pr) % NRING
                    for h in range(NH):
                        nc.tensor.matmul(
                            pvs[h][:],
                            va_view()[:, g0:g0 + 2, h, :],
                            ptv[:, s0:s0 + 2, h, :],
                            start=(pr == 0), stop=(pr == 7),
                            perf_mode=DR,
                        )

                # Software pipeline in groups of 2 k-tiles: the PE stream per
                # group is [S-pair, S-pair, PV(prev pair), filler] so the
                # S^T matmuls (which everything downstream waits on) always
                # lead, and PV lags one pair behind the exps it consumes.
                for grp in range(8):
                    for k2 in range(2):
                        kt = 2 * grp + k2
                        k0 = b * T + kt * 128
                        st = ps.tile([128, 1024], dt.float32, tag="st", bufs=2,
                                     name=f"st_{b}_{qb}_{kt}")
                        for h in range(NH):
                            nc.tensor.matmul(
                                st[:, h * 512:(h + 1) * 512],
                                kt_sb[h * HD:(h + 1) * HD, k0:k0 + 128],
                                qt_sb[h * HD:(h + 1) * HD, q0:q0 + 512],
                                start=True, stop=True,
                            )
                        slot = (qb * NKT + kt) % NRING
                        pts = pt_sb[:, slot * 1024:(slot + 1) * 1024]
                        if (kt % 16) in DVE_KT:
                            nc.vector.tensor_scalar(
                                pts.bitcast(dt.uint8), st[:],
                                SCHR_A, SCHR_B, op0=Alu.mult, op1=Alu.add,
                            )
                        else:
                            nc.scalar.activation(pts, st[:], Exp, scale=0.125)
                    if grp > 0:
                        emit_pv(grp - 1)
                    fill(2 if grp % 2 == 0 else 1)
                emit_pv(7)
                fill(100)   # drain leftover filler units

                # epilogue stage A (immediate): evacuate pv PSUM, start the
                # denominator-reshape and residual-shift DMAs
                stA = []
                for h in range(NH):
                    pv = pvs[h]
                    m = npl.tile([128, 512], dt.float32, tag="m",
                                 name=f"m_{b}_{qb}_{h}")
                    nc.vector.tensor_copy(m[:], pv[:])
                    d4 = npl.tile([128, 4], dt.float32, tag="d4",
                                  name=f"d4_{b}_{qb}_{h}")
                    nc.gpsimd.dma_start(
                        d4[:], m[HD:HD + 1, :].rearrange("p (a c) -> p a c", c=4)
                    )
                    ms = npl.tile([HD, 512], dt.float32, tag="ms",
                                  name=f"ms_{b}_{qb}_{h}")
                    nc.gpsimd.memset(ms[:], 0.0)
                    nc.gpsimd.dma_start(ms[0:HD - 1, :], m[HD + 1:128, :])
                    stA.append((h, m, d4, ms))

                # stage B (deferred into the next q-block so the d4->recip->
                # broadcast DMA latency never blocks the next exp stream)
                def stage_b(items=stA, b=b, qb=qb, q0=q0):
                    for h, m, d4, ms in items:
                        r4 = npl.tile([128, 4], dt.float32, tag="r4",
                                      name=f"r4_{b}_{qb}_{h}")
                        nc.vector.reciprocal(r4[:], d4[:])
                        rc = npl.tile([1, 512], dt.float32, tag="rc",
                                      name=f"rc_{b}_{qb}_{h}")
                        nc.gpsimd.dma_start(
                            rc[:].rearrange("p (a c) -> p a c", c=4), r4[:]
                        )
                        rbt = npl.tile([HD, 512], dt.float32, tag="rb",
                                       name=f"rb_{b}_{qb}_{h}")
                        nc.gpsimd.dma_start(
                            rbt[:],
                            rc[0:1, :].rearrange("p (o q) -> p o q", o=1)
                            .broadcast_to((1, HD, 512)),
                        )
                        otm = npl.tile([HD, 512], dt.float32, tag="otm",
                                       name=f"otm_{b}_{qb}_{h}")
                        nc.gpsimd.tensor_add(otm[:], m[0:HD, :], ms[:])
                        osl = ot_sb[h * HD:(h + 1) * HD, q0:q0 + 512]
                        nc.gpsimd.tensor_mul(osl, otm[:], rbt[:])

                if b == B - 1 and qb == NQB - 1:
                    stage_b()
                else:
                    deferred.append(stage_b)

            # ---------------- emission schedule ----------------
            for rb in range(4):
                for f in p1_block_fillers(rb):
                    emit_unit(f)
            flush_deferred()
            for b in range(B):
                for qb in range(NQB):
                    fillers = []
                    if b < B - 1:
                        fillers.extend(p1_block_fillers((b + 1) * 4 + qb))
                    if b > 0:
                        for g in range((b - 1) * 16 + qb * 4,
                                       (b - 1) * 16 + qb * 4 + 4):
                            fillers.append(lambda g=g: p3_tile(g))
                    if b == B - 1 and qb > 0:
                        for g in range(48 + (qb - 1) * 4, 48 + qb * 4):
                            fillers.append(lambda g=g: p3_tile(g))
                    p2_qblock(b, qb, fillers)
            for g in range(60, 64):
                emit_unit(lambda g=g: p3_tile(g))
            flush_deferred()
    return nc


_NC_CACHE = None


def _get_nc():
    global _NC_CACHE
    if _NC_CACHE is None:
        _NC_CACHE = build_bass()
    return _NC_CACHE


def make_in_maps(x, Wq, bq, Wk, bk, Wv, bv, Wo, bo):
    xt = np.asarray(x, dtype=np.float32).reshape(R, EMB).astype(bf16).T
    # partition-major tiles: [rb, p, kc, r] so each SBUF load is 128
    # contiguous 8KB descriptors instead of 1024 x 1KB fragments
    xt = np.ascontiguousarray(
        xt.reshape(NKC, 128, 16, 512).transpose(2, 1, 0, 3)
    ).reshape(16 * 128, NKC * 512)
    in_maps = []
    for c in range(NCORES):
        rows = slice(F * c, F * (c + 1))
        in_maps.append({
            "xt": xt,
            "wq": np.ascontiguousarray(np.asarray(Wq)[rows, :].T.astype(bf16)),
            "wk": np.ascontiguousarray(np.asarray(Wk)[rows, :].T.astype(bf16)),
            "wv": np.ascontiguousarray(np.asarray(Wv)[rows, :].T.astype(bf16)),
            "wo": np.ascontiguousarray(np.asarray(Wo)[:, rows].T.astype(bf16)),
            "bq": np.asarray(bq)[rows].reshape(F, 1).astype(np.float32),
        })
    return in_maps


def gather(results, Wv, bv, Wo, bo):
    acc = np.zeros((R, EMB), np.float32)
    for r in results:
        acc += r["out"].astype(np.float32)
    # softmax rows sum to 1, so the V-bias contributes the constant bv @ Wo^T
    acc += (np.asarray(bv, np.float64) @ np.asarray(Wo, np.float64).T
            + np.asarray(bo, np.float64)).astype(np.float32)
    return acc.reshape(B, T, EMB)


def kernel(x, Wq, bq, Wk, bk, Wv, bv, Wo, bo, _trace=False):
    nc = _get_nc()
    in_maps = make_in_maps(x, Wq, bq, Wk, bk, Wv, bv, Wo, bo)
    res = run_bass_kernel_spmd(nc, in_maps, list(range(NCORES)), trace=_trace)
    out = gather(res.results, Wv, bv, Wo, bo)
    if _trace:
        kernel.last_result = res
    return out
